# revision 25
# baseline (speedup 1.0000x reference)
"""Trainium2 Bass kernel for the gnn_message_passing problem.

Math refactor: the reference computes
    kernel[z,i,j] = einsum('zk,kij->zij', Rk*Yk, cg) * nc0[i,j]
with Rk = R @ rf_mix.T (rank 6 over paths) and Yk = Y.T @ ylm_mix.T
(rank 9 over spherical harmonics).  Rk*Yk therefore lives in a rank<=54
subspace of k, so the whole K=1024 contraction folds into a constant
    M[p*9+l, ij] = sum_k rf[k,p] * ylm_s[k,l] * cg[k,ij] * nc0[ij]
(54 x 1024, computed host-side from the constant inputs, like the other
host-side weight reshapes).  Per point the device only computes
    B[pl, z] = (R[p,z] + b2[p]) * Y'[l,z]        (radial MLP + SH)
    out[z, ij] = B[:, z].T @ M                   (k=54 fp32r matmul)

Distribution: data-parallel over z across 8 NeuronCores; constants
replicated.  Full inputs in, full output out.

Precision vs the 2e-2 gate: fp32r matmuls (11-bit mantissa, 1 cyc/row),
f16 for the tiny SH/MLP operands (10-bit mantissa), and an f16 output
staged through SBUF (halves the HBM write, which is the roofline).
Expected rel-err ~1e-3, ~20x inside the gate.

Pipeline (per 512-point group): PE transposes channel planes, computes
the radial MLP + Y broadcast + eight 512-col k-matmuls; ACT/DVE drain
PSUM to f16 SBUF; one 1 MiB DMA per group writes out.  Stages are
software-pipelined two groups deep so PE never waits on ACT/DVE.
"""

import numpy as np

import concourse.bass as bass
import concourse.tile as tile
from concourse import bacc, mybir
from concourse.bass_utils import run_bass_kernel_spmd

F32 = mybir.dt.float32
F32R = mybir.dt.float32r
F16 = mybir.dt.float16
ALU = mybir.AluOpType
ACTF = mybir.ActivationFunctionType

# Problem shape (hardcoded per contract)
Z, KDIM, DO, DI, NPATH, H = 100000, 1024, 32, 32, 6, 128
IJ = DO * DI                      # 1024
NCORES = 8
ZC = Z // NCORES                  # 12500 points per core
T = 100                           # point tiles of 128 -> ZC padded to 12800
ZC_PAD = 128 * T
TB = 4                            # tiles per group
NG = T // TB                      # 25 groups of 512 points
NCH = 10                          # channels: radii, ones, 8 scaled monomials
GZ = 128 * TB                     # 512 points per group

# Real spherical harmonic constants (l=0,1,2), folded into M host-side
C0 = 0.28209479177387814
C1 = 0.4886025119029199
C2A = 1.0925484305920792
C2B = 0.31539156525252005
C2C = 0.5462742152960396
YLM_SCALE = np.array([C0, C1, C1, C1, C2A, C2A, C2B, C2A, C2C], dtype=np.float64)

_CACHE = {}


def _build_program():
    nc = bacc.Bacc("TRN2", target_bir_lowering=False, debug=False,
                   num_devices=NCORES)

    # ---- per-core DRAM I/O ----
    rpl = nc.dram_tensor("rpl", [128, 3 * T], F32, kind="ExternalInput").ap()
    m2d = nc.dram_tensor("m2d", [118, IJ], F16, kind="ExternalInput").ap()
    # all small constants packed in one f16 blob: one DMA instead of six
    BLOB_W = TB * 128 + TB * 54 + 54 + 1 + 1 + 128      # 912
    blobd = nc.dram_tensor("blobd", [128, BLOB_W], F16, kind="ExternalInput").ap()
    out = nc.dram_tensor("out", [ZC_PAD, IJ], F16, kind="ExternalOutput").ap()

    with tile.TileContext(nc) as tc:
        with tc.tile_pool(name="const", bufs=1) as cpool, \
             tc.tile_pool(name="tps", bufs=1, space="PSUM") as tps_pool, \
             tc.tile_pool(name="hps", bufs=1, space="PSUM") as hps_pool, \
             tc.tile_pool(name="ryps", bufs=1, space="PSUM") as ry_pool, \
             tc.tile_pool(name="kps", bufs=1, space="PSUM") as kps_pool, \
             tc.tile_pool(name="tsb", bufs=3) as tpool, \
             tc.tile_pool(name="hsb", bufs=2) as hpool, \
             tc.tile_pool(name="bsb", bufs=4) as bpool, \
             tc.tile_pool(name="rbsb", bufs=2) as rbpool, \
             tc.tile_pool(name="kout", bufs=4) as kpool:

            # ---- resident constants (rpl first so phase 2 starts early) ----
            rpl_sb = cpool.tile([128, 3 * T], F32)
            nc.sync.dma_start(rpl_sb[:], rpl[:])
            blob = cpool.tile([128, BLOB_W], F16)
            nc.sync.dma_start(blob[:], blobd[:])
            m2_sb = cpool.tile([118, IJ], F16)
            nc.sync.dma_start(m2_sb[:], m2d[:])
            o = 0
            w1e_sb = blob[0:NCH * TB, o:o + TB * 128]; o += TB * 128
            ey4_sb = blob[0:NCH * TB, o:o + TB * 54]; o += TB * 54
            w2e_sb = blob[0:H, o:o + 54]; o += 54
            b1_sb = blob[0:H, o:o + 1]; o += 1
            b2_sb = blob[0:118, o:o + 1]; o += 1
            id_sb = blob[0:128, o:o + 128]; o += 128

            # =========================================================
            # Phase 2: per-point channel planes [128, T] (f16 storage):
            # radii, ones, y/r, z/r, x/r, xy/r2, yz/r2, (3zz-r2)/r2,
            # xz/r2, (xx-yy)/r2   (channel-interleaved: col = t*NCH + c)
            # =========================================================
            chan = cpool.tile([128, NCH * T], F16)
            aux = cpool.tile([128, 8 * T], F32)

            chan_v = chan[:].rearrange("p (t c) -> p c t", c=NCH)

            def phase2(c0, c1, pool=False):
                ve = nc.gpsimd if pool else nc.vector
                w = slice(c0, c1)
                x = rpl_sb[:, 0:T][:, w]
                y = rpl_sb[:, T:2 * T][:, w]
                z = rpl_sb[:, 2 * T:3 * T][:, w]

                def ax(i):
                    return aux[:, i * T:(i + 1) * T][:, w]

                xx, yy, zz, r2, inv2, va, t3, t5 = (ax(i) for i in range(8))
                ch = [chan_v[:, i, w] for i in range(NCH)]

                ve.tensor_tensor(xx, x, x, ALU.mult)
                ve.tensor_tensor(yy, y, y, ALU.mult)
                ve.tensor_tensor(zz, z, z, ALU.mult)
                ve.tensor_tensor(r2, xx, yy, ALU.add)
                ve.tensor_tensor(r2, r2, zz, ALU.add)
                nc.vector.reciprocal(inv2, r2)                   # 1/r2
                nc.scalar.sqrt(va, inv2)                         # 1/r (~7e-6)
                ve.tensor_tensor(ch[0], r2, va, ALU.mult)        # radii
                ve.memset(ch[1], 1.0)                            # l=0
                ve.tensor_tensor(ch[2], y, va, ALU.mult)         # y/r
                ve.tensor_tensor(ch[3], z, va, ALU.mult)         # z/r
                ve.tensor_tensor(ch[4], x, va, ALU.mult)         # x/r
                ve.tensor_tensor(ch[5], ch[4], ch[2], ALU.mult)
                ve.tensor_tensor(ch[6], ch[2], ch[3], ALU.mult)
                ve.tensor_scalar(t3, zz, 3.0, None, ALU.mult)
                ve.tensor_tensor(t3, t3, r2, ALU.subtract)
                ve.tensor_tensor(ch[7], t3, inv2, ALU.mult)
                ve.tensor_tensor(ch[8], ch[4], ch[3], ALU.mult)
                ve.tensor_tensor(t5, xx, yy, ALU.subtract)
                ve.tensor_tensor(ch[9], t5, inv2, ALU.mult)

            phase2(0, 2 * TB)        # groups 0-1: start the pipeline

            # =========================================================
            # Phase 3: software-pipelined main loop over 25 groups
            #   stage A(g): PE transpose -> DVE copy to SBUF f16
            #   stage B(g): radial MLP (h, relu, r) + Y broadcast + B
            #   stage C(g): 8 k-matmuls + 4 PSUM->f16 drains + 1 DMA
            # =========================================================
            # X bank: h pre-activations, time-shared with the transpose
            # output (f16 view of the first cols). Cycle per iteration:
            # h(gB) -> relu(gB) -> T(gA) -> tcopy(gA) -> h(gB+1) ...
            h_ps = hps_pool.tile([128, GZ], F32)
            t_ps = h_ps[:].bitcast(F16)[0:NCH * TB, 0:128]
            ry = ry_pool.tile([118, 512], F32)                  # r|y halves
            # quarter ring: 3 tiles of 2 k-matmuls each, reused every 3
            kq = [kps_pool.tile([128, 1024], F32, name=f"kq{i}")
                  for i in range(3)]

            # rows 54:64 of ry feed garbage lanes of the B-build; init them
            # (engine writes must start at partition 0/32/64/96)
            nc.vector.memset(ry[32:64, :], 0.0)

            t_sbs, h_sbs, b_sbs, rb_sbs, k_outs = {}, {}, {}, {}, {}

            def stage_A_pe(g):
                # transpose 4 tiles x 10 channels: [128, 40] -> [40, 128]
                nc.tensor.transpose(
                    t_ps, chan[:, NCH * TB * g: NCH * TB * (g + 1)], id_sb)

            def stage_A_dve(g):
                t_sb = tpool.tile([NCH * TB, 128], F16, tag="t_sb", name="t_sb")
                t_sbs[g] = t_sb
                nc.vector.tensor_copy(t_sb[:], t_ps)

            def stage_B_h(g):
                # hidden pre-act: h[h, z] = W1[h] * radii[z] (outer product)
                for dt in range(TB):
                    nc.tensor.matmul(
                        h_ps[:, dt * 128:(dt + 1) * 128],
                        w1e_sb[:, dt * 128:(dt + 1) * 128],
                        t_sbs[g][:], start=True, stop=True)

            def stage_B_relu(g):
                h_sb = hpool.tile([128, GZ], F16, tag="h_sb", name="h_sb")
                h_sbs[g] = h_sb
                nc.scalar.activation(h_sb[:], h_ps[:], ACTF.Relu, bias=b1_sb)

            def stage_B_r(g):
                # R[pl, z]: z halves at partition rows 0 and 64
                h_sb = h_sbs[g]
                nc.tensor.matmul(ry[0:54, 0:256], w2e_sb, h_sb[:, 0:256],
                                 start=True, stop=True)
                nc.tensor.matmul(ry[64:118, 0:256], w2e_sb, h_sb[:, 256:512],
                                 start=True, stop=True)

            def stage_B_y(g):
                # Y'[pl, z] broadcast: selection matmul per point tile
                t_sb = t_sbs[g]
                for dt in range(TB):
                    p0 = 0 if dt < 2 else 64
                    c0 = 256 + (dt % 2) * 128
                    nc.tensor.matmul(
                        ry[p0:p0 + 54, c0:c0 + 128],
                        ey4_sb[:, dt * 54:(dt + 1) * 54],
                        t_sb[:], start=True, stop=True)

            def stage_B_rb(g):
                # R + b2: ACT bias-add while draining PSUM -> SBUF f16
                rb_sb = rbpool.tile([118, 256], F16, tag="rb_sb", name="rb_sb")
                rb_sbs[g] = rb_sb
                nc.scalar.activation(rb_sb[:], ry[:, 0:256], ACTF.Identity,
                                     bias=b2_sb)

            def stage_B_b(g):
                # B = (R + b2) * Y'  (one PSUM operand, f32r SBUF out)
                b_sb = bpool.tile([118, 256], F16, tag="b_sb", name="b_sb")
                b_sbs[g] = b_sb
                nc.vector.tensor_tensor(
                    b_sb[:], rb_sbs[g][:], ry[:, 256:512], ALU.mult)

            def kmm(g, m):
                # k-matmul m = dt*2 + half; quarter q = m//2 -> ring tile
                dt, half = divmod(m, 2)
                kp = kq[(g * TB + m // 2) % 3]
                b_sb = b_sbs[g]
                p0 = 0 if dt < 2 else 64
                c0 = (dt % 2) * 128
                nc.tensor.matmul(
                    kp[:, half * 512:(half + 1) * 512],
                    b_sb[p0:p0 + 54, c0:c0 + 128],
                    m2_sb[p0:p0 + 54, half * 512:(half + 1) * 512],
                    start=True, stop=True)

            def stage_C_new_kout(g):
                k_outs[g] = kpool.tile([128, TB * IJ], F16, tag="k_out", name="k_out")

            def conv(g, q, eng):
                # drain quarter q (one ring tile) into the f16 staging tile
                src = kq[(g * TB + q) % 3]
                dest = k_outs[g][:, q * 1024:(q + 1) * 1024]
                if eng == "act":
                    nc.scalar.copy(dest, src[:])
                else:
                    nc.vector.tensor_copy(dest, src[:])

            def stage_C_dma(g):
                og = out[GZ * g:GZ * (g + 1), :].rearrange(
                    "(dt pg) ij -> pg dt ij", dt=TB)
                nc.sync.dma_start(
                    og, k_outs[g][:].rearrange("pg (dt ij) -> pg dt ij", dt=TB))

            # pipelined emission: gA = i, gB = i-1, gC = i-3 in steady
            # state (2 iters of slack for the B chain).  The prologue runs
            # groups 0 and 1 at depth 2 to start the output DMAs sooner.
            for i in range(NG + 3):
                gA, gB, gC = i, i - 1, i - 3
                vA, vB, vC = gA < NG, 0 <= gB < NG, 0 <= gC

                if vC:
                    stage_C_new_kout(gC)
                    kmm(gC, 0)
                    kmm(gC, 1)
                if vB:
                    stage_B_h(gB)
                    stage_B_relu(gB)
                if vC:
                    kmm(gC, 2)
                    kmm(gC, 3)
                if vA:
                    stage_A_pe(gA)   # X bank: after relu(gB)
                    stage_A_dve(gA)
                if vB:
                    # B chain first on ACT/DVE: it gates next group's kmms
                    stage_B_r(gB)
                    stage_B_y(gB)
                    stage_B_rb(gB)
                    stage_B_b(gB)
                if vC:
                    conv(gC, 0, "dve")
                    kmm(gC, 4)
                    kmm(gC, 5)
                    conv(gC, 1, "act")
                    kmm(gC, 6)
                    kmm(gC, 7)
                    conv(gC, 2, "dve")
                    conv(gC, 3, "act")
                    stage_C_dma(gC)
                # late phase-2 chunks ride behind the early chain ops so the
                # DVE queue never blocks group 0's tcopy/B
                if i == 0:
                    phase2(2 * TB, 6 * TB, pool=True)
                elif i == 1:
                    phase2(6 * TB, 14 * TB, pool=True)
                elif i == 2:
                    phase2(14 * TB, T, pool=True)

    nc.compile()
    return nc


def _get_program():
    if "nc" not in _CACHE:
        _CACHE["nc"] = _build_program()
    return _CACHE["nc"]


def _host_prep(r, W1, b1, W2, b2, cg, ylm_mix, rf_mix, norm_coef):
    r = np.asarray(r, dtype=np.float32)
    W1 = np.asarray(W1, dtype=np.float32)
    b1 = np.asarray(b1, dtype=np.float32)
    W2 = np.asarray(W2, dtype=np.float32)
    b2 = np.asarray(b2, dtype=np.float32)
    cg = np.asarray(cg, dtype=np.float32)
    ylm_mix = np.asarray(ylm_mix, dtype=np.float32)
    rf_mix = np.asarray(rf_mix, dtype=np.float32)
    norm_coef = np.asarray(norm_coef, dtype=np.float32)

    # constant fold: M[p*9+l, ij] = sum_k rf[k,p] ylm_s[k,l] cg[k,ij] * nc0
    ylm_s = ylm_mix.astype(np.float64) * YLM_SCALE[None, :]
    wkpl = (rf_mix.astype(np.float64)[:, :, None] * ylm_s[:, None, :])
    m54 = wkpl.reshape(KDIM, 54).T @ cg.astype(np.float64).reshape(KDIM, IJ)
    m54 *= norm_coef[:, :, 0].astype(np.float64).reshape(1, IJ)
    m2 = np.zeros((118, IJ), dtype=np.float16)
    m2[0:54] = m54.astype(np.float16)
    m2[64:118] = m2[0:54]

    # constant blob layout must match _build_program slices
    BLOB_W = TB * 128 + TB * 54 + 54 + 1 + 1 + 128
    blob = np.zeros((128, BLOB_W), dtype=np.float16)
    o = 0
    for dt in range(TB):
        blob[NCH * dt, o + dt * 128:o + (dt + 1) * 128] = W1[0].astype(np.float16)
    o += TB * 128
    for dt in range(TB):
        for l in range(9):
            for p in range(NPATH):
                blob[NCH * dt + 1 + l, o + dt * 54 + p * 9 + l] = 1.0
    o += TB * 54
    blob[0:H, o:o + 54] = np.repeat(W2, 9, axis=1).astype(np.float16)
    o += 54
    blob[0:H, o] = b1.astype(np.float16)
    o += 1
    b2r = np.repeat(b2, 9).astype(np.float16)
    blob[0:54, o] = b2r
    blob[64:118, o] = b2r
    o += 1
    blob[0:128, o:o + 128] = np.eye(128, dtype=np.float16)

    shared = {
        "m2d": m2,
        "blobd": blob,
    }

    in_maps = []
    for c in range(NCORES):
        rs = r[c * ZC:(c + 1) * ZC]
        rp = np.empty((ZC_PAD, 3), dtype=np.float32)
        rp[:ZC] = rs
        rp[ZC:] = np.array([1.0, 0.0, 0.0], dtype=np.float32)
        rpl = rp.reshape(T, 128, 3).transpose(1, 2, 0).reshape(128, 3 * T)
        m = dict(shared)
        m["rpl"] = np.ascontiguousarray(rpl)
        in_maps.append(m)
    return in_maps


def _run_device(in_maps, trace=False, **kw):
    nc = _get_program()
    return run_bass_kernel_spmd(nc, in_maps, core_ids=list(range(NCORES)),
                                trace=trace, **kw)


def kernel(r, W1, b1, W2, b2, cg, ylm_mix, rf_mix, norm_coef):
    r = np.asarray(r, dtype=np.float32)
    norm_coef_f = np.asarray(norm_coef, dtype=np.float32)
    in_maps = _host_prep(r, W1, b1, W2, b2, cg, ylm_mix, rf_mix, norm_coef_f)
    res = _run_device(in_maps)
    out = np.concatenate(
        [np.asarray(res.results[c]["out"])[:ZC] for c in range(NCORES)],
        axis=0).astype(np.float32)

    # points with exactly zero radius: recompute those rows exactly
    # (they use norm_coef[..., 1] and the safe-guarded Y)
    x, y, z = r[:, 0], r[:, 1], r[:, 2]
    r2 = (x * x + y * y) + z * z
    zero = r2 == np.float32(0.0)
    if np.any(zero):
        W1f = np.asarray(W1, np.float64)
        b1f = np.asarray(b1, np.float64)
        W2f = np.asarray(W2, np.float64)
        b2f = np.asarray(b2, np.float64)
        cgf = np.asarray(cg, np.float64)
        ylm = np.asarray(ylm_mix, np.float64)
        rf = np.asarray(rf_mix, np.float64)
        yzero = np.zeros(9); yzero[0] = C0
        hrow = np.maximum(0.0 * W1f[0] + b1f, 0.0)       # radii = 0
        rrow = hrow @ W2f + b2f
        rk = rf @ rrow                                    # [K]
        yk = ylm @ yzero                                  # [K]
        krow = np.einsum('k,kij->ij', rk * yk, cgf)
        krow = krow * np.asarray(norm_coef_f[:, :, 1], np.float64)
        out[zero] = krow.reshape(1, IJ).astype(np.float32)

    return out.reshape(Z, DO, DI)


# revision 27
# speedup vs baseline: 1.0211x; 1.0211x over previous
"""Trainium2 Bass kernel for the gnn_message_passing problem.

Math refactor: the reference computes
    kernel[z,i,j] = einsum('zk,kij->zij', Rk*Yk, cg) * nc0[i,j]
with Rk = R @ rf_mix.T (rank 6 over paths) and Yk = Y.T @ ylm_mix.T
(rank 9 over spherical harmonics).  Rk*Yk therefore lives in a rank<=54
subspace of k, so the whole K=1024 contraction folds into a constant
    M[p*9+l, ij] = sum_k rf[k,p] * ylm_s[k,l] * cg[k,ij] * nc0[ij]
(54 x 1024, computed host-side from the constant inputs, like the other
host-side weight reshapes).  Per point the device only computes
    B[pl, z] = (R[p,z] + b2[p]) * Y'[l,z]        (radial MLP + SH)
    out[z, ij] = B[:, z].T @ M                   (k=54 fp32r matmul)

Distribution: data-parallel over z across 8 NeuronCores; constants
replicated.  Full inputs in, full output out.

Precision vs the 2e-2 gate: fp32r matmuls (11-bit mantissa, 1 cyc/row),
f16 for the tiny SH/MLP operands (10-bit mantissa), and an f16 output
staged through SBUF (halves the HBM write, which is the roofline).
Expected rel-err ~1e-3, ~20x inside the gate.

Pipeline (per 512-point group): PE transposes channel planes, computes
the radial MLP + Y broadcast + eight 512-col k-matmuls; ACT/DVE drain
PSUM to f16 SBUF; one 1 MiB DMA per group writes out.  Stages are
software-pipelined two groups deep so PE never waits on ACT/DVE.
"""

import numpy as np

import concourse.bass as bass
import concourse.tile as tile
from concourse import bacc, mybir
from concourse.bass_utils import run_bass_kernel_spmd

F32 = mybir.dt.float32
F32R = mybir.dt.float32r
F16 = mybir.dt.float16
ALU = mybir.AluOpType
ACTF = mybir.ActivationFunctionType

# Problem shape (hardcoded per contract)
Z, KDIM, DO, DI, NPATH, H = 100000, 1024, 32, 32, 6, 128
IJ = DO * DI                      # 1024
NCORES = 8
ZC = Z // NCORES                  # 12500 points per core
T = 100                           # point tiles of 128 -> ZC padded to 12800
ZC_PAD = 128 * T
TB = 4                            # tiles per group
NG = T // TB                      # 25 groups of 512 points
NCH = 10                          # channels: radii, ones, 8 scaled monomials
GZ = 128 * TB                     # 512 points per group

# Real spherical harmonic constants (l=0,1,2), folded into M host-side
C0 = 0.28209479177387814
C1 = 0.4886025119029199
C2A = 1.0925484305920792
C2B = 0.31539156525252005
C2C = 0.5462742152960396
YLM_SCALE = np.array([C0, C1, C1, C1, C2A, C2A, C2B, C2A, C2C], dtype=np.float64)

_CACHE = {}


def _build_program():
    nc = bacc.Bacc("TRN2", target_bir_lowering=False, debug=False,
                   num_devices=NCORES)

    # ---- per-core DRAM I/O ----
    rpl = nc.dram_tensor("rpl", [128, 3 * T], F32, kind="ExternalInput").ap()
    m2d = nc.dram_tensor("m2d", [118, IJ], F16, kind="ExternalInput").ap()
    # all small constants packed in one f16 blob: one DMA instead of six
    BLOB_W = TB * 128 + TB * 54 + 54 + 1 + 1 + 128      # 912
    blobd = nc.dram_tensor("blobd", [128, BLOB_W], F16, kind="ExternalInput").ap()
    out = nc.dram_tensor("out", [ZC_PAD, IJ], F16, kind="ExternalOutput").ap()

    with tile.TileContext(nc) as tc:
        with tc.tile_pool(name="const", bufs=1) as cpool, \
             tc.tile_pool(name="tps", bufs=1, space="PSUM") as tps_pool, \
             tc.tile_pool(name="hps", bufs=1, space="PSUM") as hps_pool, \
             tc.tile_pool(name="ryps", bufs=1, space="PSUM") as ry_pool, \
             tc.tile_pool(name="kps", bufs=1, space="PSUM") as kps_pool, \
             tc.tile_pool(name="tsb", bufs=3) as tpool, \
             tc.tile_pool(name="hsb", bufs=2) as hpool, \
             tc.tile_pool(name="bsb", bufs=4) as bpool, \
             tc.tile_pool(name="rbsb", bufs=2) as rbpool, \
             tc.tile_pool(name="kout", bufs=4) as kpool:

            # ---- resident constants (rpl first so phase 2 starts early) ----
            rpl_sb = cpool.tile([128, 3 * T], F32)
            nc.sync.dma_start(rpl_sb[:], rpl[:])
            blob = cpool.tile([128, BLOB_W], F16)
            nc.sync.dma_start(blob[:], blobd[:])
            m2_sb = cpool.tile([118, IJ], F16)
            nc.sync.dma_start(m2_sb[:], m2d[:])
            o = 0
            w1e_sb = blob[0:NCH * TB, o:o + TB * 128]; o += TB * 128
            ey4_sb = blob[0:NCH * TB, o:o + TB * 54]; o += TB * 54
            w2e_sb = blob[0:H, o:o + 54]; o += 54
            b1_sb = blob[0:H, o:o + 1]; o += 1
            b2_sb = blob[0:118, o:o + 1]; o += 1
            id_sb = blob[0:128, o:o + 128]; o += 128

            # =========================================================
            # Phase 2: per-point channel planes [128, T] (f16 storage):
            # radii, ones, y/r, z/r, x/r, xy/r2, yz/r2, (3zz-r2)/r2,
            # xz/r2, (xx-yy)/r2   (channel-interleaved: col = t*NCH + c)
            # =========================================================
            chan = cpool.tile([128, NCH * T], F16)
            aux = cpool.tile([128, 8 * T], F32)

            chan_v = chan[:].rearrange("p (t c) -> p c t", c=NCH)

            def phase2_base():
                # full-width radial quantities: r2, 1/r2, 1/r (one pass)
                x = rpl_sb[:, 0:T]
                y = rpl_sb[:, T:2 * T]
                z = rpl_sb[:, 2 * T:3 * T]
                xx, yy, zz, r2, inv2, va, t3, t5 = (
                    aux[:, i * T:(i + 1) * T] for i in range(8))
                nc.vector.tensor_tensor(xx, x, x, ALU.mult)
                nc.vector.tensor_tensor(yy, y, y, ALU.mult)
                nc.vector.tensor_tensor(zz, z, z, ALU.mult)
                nc.vector.tensor_tensor(r2, xx, yy, ALU.add)
                nc.vector.tensor_tensor(r2, r2, zz, ALU.add)
                nc.vector.reciprocal(inv2, r2)                   # 1/r2
                nc.scalar.sqrt(va, inv2)                         # 1/r (~7e-6)

            def phase2(c0, c1, pool=False):
                ve = nc.gpsimd if pool else nc.vector
                w = slice(c0, c1)
                POOL = pool
                x = rpl_sb[:, 0:T][:, w]
                y = rpl_sb[:, T:2 * T][:, w]
                z = rpl_sb[:, 2 * T:3 * T][:, w]

                def ax(i):
                    return aux[:, i * T:(i + 1) * T][:, w]

                xx, yy, zz, r2, inv2, va, t3, t5 = (ax(i) for i in range(8))
                ch = [chan_v[:, i, w] for i in range(NCH)]

                ve.memset(ch[1], 1.0)                            # l=0
                ve.tensor_tensor(ch[0], r2, va, ALU.mult)        # radii
                ve.tensor_tensor(ch[2], y, va, ALU.mult)         # y/r
                ve.tensor_tensor(ch[3], z, va, ALU.mult)         # z/r
                ve.tensor_tensor(ch[4], x, va, ALU.mult)         # x/r
                ve.tensor_tensor(ch[5], ch[4], ch[2], ALU.mult)
                ve.tensor_tensor(ch[6], ch[2], ch[3], ALU.mult)
                ve.tensor_scalar(t3, zz, 3.0, None, ALU.mult)
                ve.tensor_tensor(t3, t3, r2, ALU.subtract)
                ve.tensor_tensor(ch[7], t3, inv2, ALU.mult)
                ve.tensor_tensor(ch[8], ch[4], ch[3], ALU.mult)
                ve.tensor_tensor(t5, xx, yy, ALU.subtract)
                ve.tensor_tensor(ch[9], t5, inv2, ALU.mult)

            phase2_base()
            phase2(0, 2 * TB)        # groups 0-1: start the pipeline

            # =========================================================
            # Phase 3: software-pipelined main loop over 25 groups
            #   stage A(g): PE transpose -> DVE copy to SBUF f16
            #   stage B(g): radial MLP (h, relu, r) + Y broadcast + B
            #   stage C(g): 8 k-matmuls + 4 PSUM->f16 drains + 1 DMA
            # =========================================================
            # X bank: h pre-activations, time-shared with the transpose
            # output (f16 view of the first cols). Cycle per iteration:
            # h(gB) -> relu(gB) -> T(gA) -> tcopy(gA) -> h(gB+1) ...
            h_ps = hps_pool.tile([128, GZ], F32)
            t_ps = h_ps[:].bitcast(F16)[0:NCH * TB, 0:128]
            ry = ry_pool.tile([118, 512], F32)                  # r|y halves
            # quarter ring: 3 tiles of 2 k-matmuls each, reused every 3
            kq = [kps_pool.tile([128, 1024], F32, name=f"kq{i}")
                  for i in range(3)]

            # rows 54:64 of ry feed garbage lanes of the B-build; init them
            # (engine writes must start at partition 0/32/64/96)
            nc.vector.memset(ry[32:64, :], 0.0)

            t_sbs, h_sbs, b_sbs, rb_sbs, k_outs = {}, {}, {}, {}, {}

            def stage_A_pe(g):
                # transpose 4 tiles x 10 channels: [128, 40] -> [40, 128]
                nc.tensor.transpose(
                    t_ps, chan[:, NCH * TB * g: NCH * TB * (g + 1)], id_sb)

            def stage_A_dve(g):
                t_sb = tpool.tile([NCH * TB, 128], F16, tag="t_sb", name="t_sb")
                t_sbs[g] = t_sb
                nc.vector.tensor_copy(t_sb[:], t_ps)

            def stage_B_h(g):
                # hidden pre-act: h[h, z] = W1[h] * radii[z] (outer product)
                for dt in range(TB):
                    nc.tensor.matmul(
                        h_ps[:, dt * 128:(dt + 1) * 128],
                        w1e_sb[:, dt * 128:(dt + 1) * 128],
                        t_sbs[g][:], start=True, stop=True)

            def stage_B_relu(g):
                h_sb = hpool.tile([128, GZ], F16, tag="h_sb", name="h_sb")
                h_sbs[g] = h_sb
                nc.scalar.activation(h_sb[:], h_ps[:], ACTF.Relu, bias=b1_sb)

            def stage_B_r(g):
                # R[pl, z]: z halves at partition rows 0 and 64
                h_sb = h_sbs[g]
                nc.tensor.matmul(ry[0:54, 0:256], w2e_sb, h_sb[:, 0:256],
                                 start=True, stop=True)
                nc.tensor.matmul(ry[64:118, 0:256], w2e_sb, h_sb[:, 256:512],
                                 start=True, stop=True)

            def stage_B_y(g):
                # Y'[pl, z] broadcast: selection matmul per point tile
                t_sb = t_sbs[g]
                for dt in range(TB):
                    p0 = 0 if dt < 2 else 64
                    c0 = 256 + (dt % 2) * 128
                    nc.tensor.matmul(
                        ry[p0:p0 + 54, c0:c0 + 128],
                        ey4_sb[:, dt * 54:(dt + 1) * 54],
                        t_sb[:], start=True, stop=True)

            def stage_B_rb(g):
                # R + b2: ACT bias-add while draining PSUM -> SBUF f16
                rb_sb = rbpool.tile([118, 256], F16, tag="rb_sb", name="rb_sb")
                rb_sbs[g] = rb_sb
                nc.scalar.activation(rb_sb[:], ry[:, 0:256], ACTF.Identity,
                                     bias=b2_sb)

            def stage_B_b(g):
                # B = (R + b2) * Y'  (one PSUM operand, f32r SBUF out)
                b_sb = bpool.tile([118, 256], F16, tag="b_sb", name="b_sb")
                b_sbs[g] = b_sb
                nc.vector.tensor_tensor(
                    b_sb[:], rb_sbs[g][:], ry[:, 256:512], ALU.mult)

            def kmm(g, m):
                # k-matmul m = dt*2 + half; quarter q = m//2 -> ring tile
                dt, half = divmod(m, 2)
                kp = kq[(g * TB + m // 2) % 3]
                b_sb = b_sbs[g]
                p0 = 0 if dt < 2 else 64
                c0 = (dt % 2) * 128
                nc.tensor.matmul(
                    kp[:, half * 512:(half + 1) * 512],
                    b_sb[p0:p0 + 54, c0:c0 + 128],
                    m2_sb[p0:p0 + 54, half * 512:(half + 1) * 512],
                    start=True, stop=True)

            def stage_C_new_kout(g):
                k_outs[g] = kpool.tile([128, TB * IJ], F16, tag="k_out", name="k_out")

            def conv(g, q, eng):
                # drain quarter q (one ring tile) into the f16 staging tile
                src = kq[(g * TB + q) % 3]
                dest = k_outs[g][:, q * 1024:(q + 1) * 1024]
                if eng == "act":
                    nc.scalar.copy(dest, src[:])
                else:
                    nc.vector.tensor_copy(dest, src[:])

            def stage_C_dma(g):
                og = out[GZ * g:GZ * (g + 1), :].rearrange(
                    "(dt pg) ij -> pg dt ij", dt=TB)
                nc.sync.dma_start(
                    og, k_outs[g][:].rearrange("pg (dt ij) -> pg dt ij", dt=TB))

            # pipelined emission: gA = i, gB = i-1, gC = i-3 in steady
            # state (2 iters of slack for the B chain).  The prologue runs
            # groups 0 and 1 at depth 2 to start the output DMAs sooner.
            for i in range(NG + 3):
                gA, gB, gC = i, i - 1, i - 3
                vA, vB, vC = gA < NG, 0 <= gB < NG, 0 <= gC

                if vC:
                    stage_C_new_kout(gC)
                    kmm(gC, 0)
                    kmm(gC, 1)
                if vB:
                    stage_B_h(gB)
                    stage_B_relu(gB)
                if vC:
                    kmm(gC, 2)
                    kmm(gC, 3)
                if vA:
                    stage_A_pe(gA)   # X bank: after relu(gB)
                    stage_A_dve(gA)
                if vB:
                    # B chain first on ACT/DVE: it gates next group's kmms
                    stage_B_r(gB)
                    stage_B_y(gB)
                    stage_B_rb(gB)
                    stage_B_b(gB)
                if vC:
                    conv(gC, 0, "dve")
                    kmm(gC, 4)
                    kmm(gC, 5)
                    conv(gC, 1, "act")
                    kmm(gC, 6)
                    kmm(gC, 7)
                    conv(gC, 2, "dve")
                    conv(gC, 3, "act")
                    stage_C_dma(gC)
                # late phase-2 chunks ride behind the early chain ops so the
                # DVE queue never blocks group 0's tcopy/B
                if i == 0:
                    phase2(2 * TB, 6 * TB, pool=True)
                elif i == 1:
                    phase2(6 * TB, 14 * TB, pool=True)
                elif i == 2:
                    phase2(14 * TB, T, pool=True)

    nc.compile()
    return nc


def _get_program():
    if "nc" not in _CACHE:
        _CACHE["nc"] = _build_program()
    return _CACHE["nc"]


def _host_prep(r, W1, b1, W2, b2, cg, ylm_mix, rf_mix, norm_coef):
    r = np.asarray(r, dtype=np.float32)
    W1 = np.asarray(W1, dtype=np.float32)
    b1 = np.asarray(b1, dtype=np.float32)
    W2 = np.asarray(W2, dtype=np.float32)
    b2 = np.asarray(b2, dtype=np.float32)
    cg = np.asarray(cg, dtype=np.float32)
    ylm_mix = np.asarray(ylm_mix, dtype=np.float32)
    rf_mix = np.asarray(rf_mix, dtype=np.float32)
    norm_coef = np.asarray(norm_coef, dtype=np.float32)

    # constant fold: M[p*9+l, ij] = sum_k rf[k,p] ylm_s[k,l] cg[k,ij] * nc0
    ylm_s = ylm_mix.astype(np.float64) * YLM_SCALE[None, :]
    wkpl = (rf_mix.astype(np.float64)[:, :, None] * ylm_s[:, None, :])
    m54 = wkpl.reshape(KDIM, 54).T @ cg.astype(np.float64).reshape(KDIM, IJ)
    m54 *= norm_coef[:, :, 0].astype(np.float64).reshape(1, IJ)
    m2 = np.zeros((118, IJ), dtype=np.float16)
    m2[0:54] = m54.astype(np.float16)
    m2[64:118] = m2[0:54]

    # constant blob layout must match _build_program slices
    BLOB_W = TB * 128 + TB * 54 + 54 + 1 + 1 + 128
    blob = np.zeros((128, BLOB_W), dtype=np.float16)
    o = 0
    for dt in range(TB):
        blob[NCH * dt, o + dt * 128:o + (dt + 1) * 128] = W1[0].astype(np.float16)
    o += TB * 128
    for dt in range(TB):
        for l in range(9):
            for p in range(NPATH):
                blob[NCH * dt + 1 + l, o + dt * 54 + p * 9 + l] = 1.0
    o += TB * 54
    blob[0:H, o:o + 54] = np.repeat(W2, 9, axis=1).astype(np.float16)
    o += 54
    blob[0:H, o] = b1.astype(np.float16)
    o += 1
    b2r = np.repeat(b2, 9).astype(np.float16)
    blob[0:54, o] = b2r
    blob[64:118, o] = b2r
    o += 1
    blob[0:128, o:o + 128] = np.eye(128, dtype=np.float16)

    shared = {
        "m2d": m2,
        "blobd": blob,
    }

    in_maps = []
    for c in range(NCORES):
        rs = r[c * ZC:(c + 1) * ZC]
        rp = np.empty((ZC_PAD, 3), dtype=np.float32)
        rp[:ZC] = rs
        rp[ZC:] = np.array([1.0, 0.0, 0.0], dtype=np.float32)
        rpl = rp.reshape(T, 128, 3).transpose(1, 2, 0).reshape(128, 3 * T)
        m = dict(shared)
        m["rpl"] = np.ascontiguousarray(rpl)
        in_maps.append(m)
    return in_maps


def _run_device(in_maps, trace=False, **kw):
    nc = _get_program()
    return run_bass_kernel_spmd(nc, in_maps, core_ids=list(range(NCORES)),
                                trace=trace, **kw)


def kernel(r, W1, b1, W2, b2, cg, ylm_mix, rf_mix, norm_coef):
    r = np.asarray(r, dtype=np.float32)
    norm_coef_f = np.asarray(norm_coef, dtype=np.float32)
    in_maps = _host_prep(r, W1, b1, W2, b2, cg, ylm_mix, rf_mix, norm_coef_f)
    res = _run_device(in_maps)
    out = np.concatenate(
        [np.asarray(res.results[c]["out"])[:ZC] for c in range(NCORES)],
        axis=0).astype(np.float32)

    # points with exactly zero radius: recompute those rows exactly
    # (they use norm_coef[..., 1] and the safe-guarded Y)
    x, y, z = r[:, 0], r[:, 1], r[:, 2]
    r2 = (x * x + y * y) + z * z
    zero = r2 == np.float32(0.0)
    if np.any(zero):
        W1f = np.asarray(W1, np.float64)
        b1f = np.asarray(b1, np.float64)
        W2f = np.asarray(W2, np.float64)
        b2f = np.asarray(b2, np.float64)
        cgf = np.asarray(cg, np.float64)
        ylm = np.asarray(ylm_mix, np.float64)
        rf = np.asarray(rf_mix, np.float64)
        yzero = np.zeros(9); yzero[0] = C0
        hrow = np.maximum(0.0 * W1f[0] + b1f, 0.0)       # radii = 0
        rrow = hrow @ W2f + b2f
        rk = rf @ rrow                                    # [K]
        yk = ylm @ yzero                                  # [K]
        krow = np.einsum('k,kij->ij', rk * yk, cgf)
        krow = krow * np.asarray(norm_coef_f[:, :, 1], np.float64)
        out[zero] = krow.reshape(1, IJ).astype(np.float32)

    return out.reshape(Z, DO, DI)
